# revision 1
# baseline (speedup 1.0000x reference)
"""Trainium2 Bass kernel for nn_BobaTransformerBlock (dense transformer block
with linear attention + poly-gelu MLP), data-parallel over batch on 8 cores.

Math (per sample, exact reassociation of the reference):
  h  = x * g1
  Gx = x^T x                                   [256,256]   (device, streamed)
  per head h: KV_h = wk'_h Gx wv'_h^T ; MT_h = KV_h^T wq'_h   (wX' = wX * g1)
  P  = (SCALE/N) * M @ w_out^T ;  PI = P + I
  x2 = x @ PI + b_out                          (attention + residual)
  m  = x2 @ w1g^T + b1                         (w1g = w1 * g2)
  poly_gelu(m) = 0.5m + 0.1972 m^3 + 0.0012 m^4
  y  = x2 @ Wlin^T + nl @ w2^T + B2
       where Wlin = I + 0.5 w2 @ w1g,  B2 = b2 + 0.5 w2 @ b1,
             nl   = (0.0012 m^2 + 0.1972 m) * m^2   (computed on-chip in bf16)

Device layout is channel-major ("transposed"): activations [c, n] so biases are
per-partition. Host supplies x both natural (for the Gram phase) and
transposed (for everything else); the device writes y transposed and the host
transposes back during unsharding.
"""

import sys

for _p in ("/opt/trn_rl_repo", "/opt/pypackages"):
    if _p not in sys.path:
        sys.path.insert(0, _p)

from contextlib import ExitStack

import numpy as np

import concourse.bass as bass
import concourse.mybir as mybir
import concourse.tile as tile
from concourse.bass_utils import run_bass_kernel_spmd

F32 = mybir.dt.float32
F32R = mybir.dt.float32r
BF16 = mybir.dt.bfloat16
NP_BF16 = mybir.dt.np(BF16)
AF = mybir.ActivationFunctionType
ALU = mybir.AluOpType

B, N, C = 8, 8192, 256
H, D = 8, 64
INNER = H * D          # 512
MLP = 4 * C            # 1024
SCALE = 1.0 / np.sqrt(D)
S_ATTN = float(SCALE / N)
N_CORES = 8
NT2 = N // 512         # phase-2 macro tiles

# which MLP hidden chunks compute m^2 on the scalar engine (rest on vector)
ACT_SQUARE_CHUNKS = (1,)  # pair indices whose m^2 runs on ScalarE

_NC = None             # cached Bass program
LAST_RESULTS = None    # BassKernelResults of the most recent run (for test.py)


def _legalize_waits(nc, max_waits=1):
    """walrus's TPB codegen accepts at most one sync wait per instruction.
    Move excess waits onto preceding same-engine NOPs."""
    ctr = 0
    for f in nc.m.functions:
        for bb in f.blocks:
            insts = bb.instructions
            i = 0
            while i < len(insts):
                inst = insts[i]
                si = inst.sync_info
                waits = list(si.on_wait) if (si is not None and si.on_wait) else []
                if len(waits) > max_waits:
                    keep = waits[-max_waits:]
                    extra = waits[:-max_waits]
                    pos = i
                    while extra:
                        chunk, extra = extra[:max_waits], extra[max_waits:]
                        nop = mybir.InstNoOp(
                            name=f"I-waitsplit-{ctr}",
                            engine=inst.engine,
                            ins=[],
                            outs=[],
                            sync_info=mybir.SyncInfo(on_wait=chunk, on_update=[]),
                        )
                        ctr += 1
                        insts.insert(pos, nop)
                        pos += 1
                        i += 1
                    inst.sync_info = mybir.SyncInfo(
                        on_wait=keep,
                        on_update=list(si.on_update) if si.on_update else [],
                    )
                i += 1
    return ctr


def _build_program(reps=1):
    nc = bass.Bass(trn_type="TRN2")

    x_d = nc.declare_dram_parameter("x", [N, C], BF16, isOutput=False)
    xt_d = nc.declare_dram_parameter("xt", [C, N], F32R, isOutput=False)
    wkv_d = nc.declare_dram_parameter("wkv", [128, 2, 2 * INNER], F32R, isOutput=False)
    wq_d = nc.declare_dram_parameter("wq", [64, H, C], F32R, isOutput=False)
    wo_d = nc.declare_dram_parameter("wo", [128, 4, C], F32R, isOutput=False)
    w1t_d = nc.declare_dram_parameter("w1t", [128, 2, MLP], F32R, isOutput=False)
    w2t_d = nc.declare_dram_parameter("w2t", [128, 8, C], BF16, isOutput=False)
    wlin_d = nc.declare_dram_parameter("wlin", [128, 2, C], F32R, isOutput=False)
    ident_d = nc.declare_dram_parameter("ident", [128, 2, C], F32, isOutput=False)
    bias_d = nc.declare_dram_parameter("bias", [128, 12], F32, isOutput=False)
    yt_d = nc.declare_dram_parameter("yt", [2, 128, N], F32, isOutput=True)

    def r(ap):
        return ap if ap.dtype == F32R else ap.bitcast(F32R)

    with tile.TileContext(nc) as tc, ExitStack() as ctx:
        const = ctx.enter_context(tc.tile_pool(name="const", bufs=1))
        wkv = const.tile([128, 2, 2 * INNER], F32R, name="wkv", tag="wkv")
        wq = const.tile([64, H, C], F32R, name="wq", tag="wq")
        wo = const.tile([128, 4, C], F32R, name="wo", tag="wo")
        w1t = const.tile([128, 2, MLP], F32R, name="w1t", tag="w1t")
        w2t = const.tile([128, 8, C], BF16, name="w2t", tag="w2t")
        wlin = const.tile([128, 2, C], F32R, name="wlin", tag="wlin")
        ident = const.tile([128, 2, C], F32, name="ident", tag="ident")
        bias = const.tile([128, 12], F32, name="bias", tag="bias")
        PI = const.tile([128, 2, C], F32R, name="PI", tag="PI")
        G_sb = const.tile([128, 2, C], F32R, name="G", tag="G")
        MT_sb = const.tile([128, 4, C], F32R, name="MT", tag="MT")
        xt_res = const.tile([128, 2, N], F32R, name="xt_res", tag="xt_res")
        PIT = const.tile([128, 2, C], F32R, name="PIT", tag="PIT")
        W1F = const.tile([128, 2, MLP], F32R, name="W1F", tag="W1F")
        WLF = const.tile([128, 2, C], F32R, name="WLF", tag="WLF")

        for _rep in range(reps):

            # ---------------- Phase 1: Gram matrix Gx = x^T x ----------------
            with tc.tile_pool(name="xp", bufs=4) as xp, \
                 tc.tile_pool(name="gps", bufs=1, space="PSUM") as gps:
                g_ps = [gps.tile([128, C], F32, name=f"g{k}", tag=f"g{k}") for k in range(2)]
                n_big = N // 1024
                for tb in range(n_big):
                    x_t = xp.tile([128, 8, C], BF16, name="x", tag="x")
                    if tb == 0:
                        for hh in range(4):
                            nc.sync.dma_start(
                                out=x_t[:, hh * 2:(hh + 1) * 2, :],
                                in_=x_d[hh * 256:(hh + 1) * 256, :].rearrange(
                                    "(a p) c -> p a c", p=128),
                            )
                    else:
                        nc.sync.dma_start(
                            out=x_t[:],
                            in_=x_d[tb * 1024:(tb + 1) * 1024, :].rearrange(
                                "(a p) c -> p a c", p=128),
                        )
                    for a in range(8):
                        for k in range(2):
                            nc.tensor.matmul(
                                g_ps[k][:],
                                lhsT=x_t[:, a, k * 128:(k + 1) * 128],
                                rhs=x_t[:, a, :],
                                start=(tb == 0 and a == 0),
                                stop=(tb == n_big - 1 and a == 7),
                            )
                for k in range(2):
                    nc.scalar.activation(out=G_sb[:, k, :], in_=g_ps[k][:], func=AF.Copy)

            # Ordered input stream on the SP HWDGE ring (just-in-time): the Gram
            # x chunks were emitted above; now the phase-1.5 weights, the first
            # xt quarter, then phase-2 weights, then the remaining xt quarters.
            for sb, dr in ((wkv, wkv_d), (wq, wq_d), (wo, wo_d), (ident, ident_d)):
                nc.sync.dma_start(out=sb[:], in_=dr[:])
            for k in range(2):
                nc.sync.dma_start(out=xt_res[:, k, 0:2048], in_=xt_d[k * 128:(k + 1) * 128, 0:2048])
            for sb, dr in ((w1t, w1t_d), (w2t, w2t_d), (wlin, wlin_d), (bias, bias_d)):
                nc.sync.dma_start(out=sb[:], in_=dr[:])
            for J in range(1, 4):
                for k in range(2):
                    nc.sync.dma_start(
                        out=xt_res[:, k, J * 2048:(J + 1) * 2048],
                        in_=xt_d[k * 128:(k + 1) * 128, J * 2048:(J + 1) * 2048])

            # ---------------- Phase 1.5: per-head KV path -> PI ----------------
            with tc.tile_pool(name="hsb", bufs=6) as hsb, \
                 tc.tile_pool(name="hps", bufs=6, space="PSUM") as hps, \
                 tc.tile_pool(name="pps", bufs=1, space="PSUM") as pps:
                # ATall = Gx @ wk'^T for all heads at once (Gx is symmetric, so
                # no transpose of the intermediate is ever needed)
                atall = hsb.tile([128, 2, INNER], F32R, name="atall", tag="atall")
                for cc in range(2):
                    at_ps = hps.tile([128, INNER], F32, name="hps", tag="hps")
                    for k2 in range(2):
                        nc.tensor.matmul(
                            at_ps[:],
                            lhsT=r(G_sb[:, k2, cc * 128:(cc + 1) * 128]),
                            rhs=r(wkv[:, k2, 0:INNER]),
                            start=(k2 == 0), stop=(k2 == 1),
                        )
                    nc.scalar.activation(out=atall[:, cc, :], in_=at_ps[:], func=AF.Copy)

                for h in range(H):
                    kv_ps = hps.tile([64, 64], F32, name="hps", tag="hps")
                    for kk in range(2):
                        nc.tensor.matmul(
                            kv_ps[:],
                            lhsT=atall[:, kk, h * 64:(h + 1) * 64],
                            rhs=r(wkv[:, kk, INNER + h * 64:INNER + (h + 1) * 64]),
                            start=(kk == 0), stop=(kk == 1),
                        )
                    kv_sb = hsb.tile([64, 64], F32R, name="kv", tag="kv")
                    nc.scalar.activation(out=kv_sb[:], in_=kv_ps[:], func=AF.Copy)

                    mt_ps = hps.tile([64, C], F32, name="hps", tag="hps")
                    nc.tensor.matmul(mt_ps[:], lhsT=r(kv_sb[:]), rhs=r(wq[:, h, :]),
                                     start=True, stop=True)
                    nc.scalar.activation(
                        out=MT_sb[(h % 2) * 64:(h % 2 + 1) * 64, h // 2, :],
                        in_=mt_ps[:], func=AF.Copy,
                    )

                for cc in range(2):
                    p_ps = pps.tile([128, C], F32, name=f"p{cc}", tag=f"p{cc}")
                    for kk in range(4):
                        nc.tensor.matmul(
                            p_ps[:],
                            lhsT=r(MT_sb[:, kk, cc * 128:(cc + 1) * 128]),
                            rhs=r(wo[:, kk, :]),
                            start=(kk == 0), stop=(kk == 3),
                        )
                    # PI = P * S_ATTN + I
                    nc.vector.scalar_tensor_tensor(
                        out=PI[:, cc, :], in0=p_ps[:], scalar=S_ATTN,
                        in1=ident[:, cc, :], op0=ALU.mult, op1=ALU.add,
                    )

                # PIT = PI^T (so PI can be the contraction-side operand)
                for i in range(2):
                    for kb in range(2):
                        pit_ps = hps.tile([128, 128], F32, name="hps", tag="hps")
                        nc.tensor.transpose(
                            pit_ps[:],
                            PI[:, i, kb * 128:(kb + 1) * 128].bitcast(F32),
                            ident[:, 0, 0:128],
                        )
                        nc.scalar.activation(out=PIT[:, kb, i * 128:(i + 1) * 128],
                                             in_=pit_ps[:], func=AF.Copy)
                # W1F = PI @ w1g^T and WLF = PI @ Wlin^T: fold the attention
                # apply into the MLP/output weights so x2 is never materialized.
                for cb in range(2):
                    for oh in range(2):
                        wf_ps = hps.tile([128, 512], F32, name="wf", tag="hps")
                        for k2 in range(2):
                            nc.tensor.matmul(
                                wf_ps[:],
                                lhsT=PIT[:, k2, cb * 128:(cb + 1) * 128],
                                rhs=w1t[:, k2, oh * 512:(oh + 1) * 512],
                                start=(k2 == 0), stop=(k2 == 1),
                            )
                        nc.scalar.activation(
                            out=W1F[:, cb, oh * 512:(oh + 1) * 512],
                            in_=wf_ps[:], func=AF.Copy)
                    wl_ps = hps.tile([128, C], F32, name="wl", tag="hps")
                    for k2 in range(2):
                        nc.tensor.matmul(
                            wl_ps[:],
                            lhsT=PIT[:, k2, cb * 128:(cb + 1) * 128],
                            rhs=wlin[:, k2, :],
                            start=(k2 == 0), stop=(k2 == 1),
                        )
                    nc.scalar.activation(out=WLF[:, cb, :], in_=wl_ps[:], func=AF.Copy)

            # ---------------- Phase 2: streamed MLP (attention pre-folded) ----------------
            with tc.tile_pool(name="gel", bufs=3) as gel, \
                 tc.tile_pool(name="nlp", bufs=2) as nlp, \
                 tc.tile_pool(name="yp", bufs=2) as yp, \
                 tc.tile_pool(name="mps", bufs=6, space="PSUM") as mps, \
                 tc.tile_pool(name="yps", bufs=2, space="PSUM") as yps:
                def emit_y(j, nl):
                    # y = x @ WLF + nl @ w2^T + B2   (deferred one iteration
                    # so PE always has MLP1 work while DVE finishes the gelu chain)
                    sl = slice(j * 512, (j + 1) * 512)
                    for cc in range(2):
                        y_ps = yps.tile([128, 512], F32, name="y", tag="y")
                        for k in range(2):
                            nc.tensor.matmul(
                                y_ps[:],
                                lhsT=WLF[:, k, cc * 128:(cc + 1) * 128],
                                rhs=xt_res[:, k, sl],
                                start=(k == 0), stop=False,
                            )
                        for kk in range(8):
                            nc.tensor.matmul(
                                y_ps[:],
                                lhsT=w2t[:, kk, cc * 128:(cc + 1) * 128],
                                rhs=nl[:, kk, :],
                                start=False, stop=(kk == 7),
                            )
                        y_sb = yp.tile([128, 512], F32, name=f"y{cc}", tag=f"y{cc}")
                        nc.scalar.activation(out=y_sb[:], in_=y_ps[:], func=AF.Identity,
                                             bias=bias[:, 10 + cc:11 + cc])
                        if j >= NT2 - 2:
                            nc.sync.dma_start(out=yt_d[cc, :, sl], in_=y_sb[:])
                        else:
                            nc.gpsimd.dma_start(out=yt_d[cc, :, sl], in_=y_sb[:])

                pending = []
                for j in range(NT2):
                    sl = slice(j * 512, (j + 1) * 512)

                    # MLP hidden + poly-gelu nonlinear part (bf16 elementwise,
                    # o-chunks processed in pairs so DVE ops run at [128, 1024])
                    nl = nlp.tile([128, 8, 512], BF16, name="nl", tag="nl")
                    for op_ in range(4):
                        mb = gel.tile([128, 2, 512], BF16, name="mb", tag="mb")
                        for half in range(2):
                            o = op_ * 2 + half
                            m_ps = mps.tile([128, 512], F32, name="m", tag="m")
                            for k in range(2):
                                nc.tensor.matmul(
                                    m_ps[:],
                                    lhsT=W1F[:, k, o * 128:(o + 1) * 128],
                                    rhs=xt_res[:, k, sl],
                                    start=(k == 0), stop=(k == 1),
                                )
                            nc.scalar.activation(out=mb[:, half, :], in_=m_ps[:],
                                                 func=AF.Identity,
                                                 bias=bias[:, 2 + o:3 + o])
                        st = gel.tile([128, 2, 512], BF16, name="st", tag="st")
                        st_eng = nc.gpsimd if op_ in (0, 1, 2) else nc.vector
                        st_eng.tensor_scalar(out=st[:], in0=mb[:],
                                             scalar1=0.0012, scalar2=0.1972,
                                             op0=ALU.mult, op1=ALU.add)
                        p2 = gel.tile([128, 2, 512], BF16, name="p2", tag="p2")
                        nc.vector.tensor_tensor(out=p2[:], in0=mb[:], in1=st[:], op=ALU.mult)
                        wsq = gel.tile([128, 2, 512], BF16, name="wsq", tag="wsq")
                        if op_ in ACT_SQUARE_CHUNKS:
                            nc.scalar.activation(out=wsq[:], in_=mb[:], func=AF.Square)
                        else:
                            nc.vector.tensor_tensor(out=wsq[:], in0=mb[:], in1=mb[:], op=ALU.mult)
                        nc.vector.tensor_tensor(out=nl[:, op_ * 2:op_ * 2 + 2, :],
                                                in0=p2[:], in1=wsq[:], op=ALU.mult)

                    pending.append((j, nl))
                    if len(pending) > 1:
                        emit_y(*pending.pop(0))
                for pj in pending:
                    emit_y(*pj)

    _legalize_waits(nc, 1)
    return nc


def _get_program(reps=1):
    global _NC
    if reps != 1:
        return _build_program(reps)
    if _NC is None:
        _NC = _build_program()
    return _NC


def _prep_maps(x, gamma1, w_qkv, w_out, b_out, gamma2, w1, b1, w2, b2):
    f8 = np.float64
    x = np.asarray(x, np.float32)
    g1 = np.asarray(gamma1, f8)
    g2 = np.asarray(gamma2, f8)
    w_qkv = np.asarray(w_qkv, f8)
    w_out = np.asarray(w_out, f8)
    b_out = np.asarray(b_out, f8)
    w1 = np.asarray(w1, f8)
    b1 = np.asarray(b1, f8)
    w2 = np.asarray(w2, f8)
    b2 = np.asarray(b2, f8)

    wq = w_qkv[0:INNER] * g1[None, :]
    wk = w_qkv[INNER:2 * INNER] * g1[None, :]
    wv = w_qkv[2 * INNER:3 * INNER] * g1[None, :]
    w1g = w1 * g2[None, :]
    wlin_m = np.eye(C) + 0.5 * (w2 @ w1g)       # [c', c]
    b2v = b2 + 0.5 * (w2 @ b1)

    def pk(a, kdim):  # [kdim*128, F] -> [128, kdim, F]
        return np.ascontiguousarray(
            a.reshape(kdim, 128, a.shape[-1]).transpose(1, 0, 2)).astype(np.float32)

    wkvT = np.concatenate([wk.T, wv.T], axis=1)             # [256, 1024]
    wkv_h = pk(wkvT, 2)
    wq_h = np.ascontiguousarray(
        wq.reshape(H, 64, C).transpose(1, 0, 2)).astype(np.float32)
    wo_h = pk(w_out.T.copy(), 4)                            # [512,256]->[128,4,256]
    w1t_h = pk(w1g.T.copy(), 2)                             # [256,1024]->[128,2,1024]
    w2t_h = np.ascontiguousarray(
        w2.T.reshape(8, 128, C).transpose(1, 0, 2)).astype(NP_BF16)
    wlin_h = pk(wlin_m.T.copy(), 2)                         # [256,256]->[128,2,256]
    ident_h = pk(np.eye(C), 2)
    b1f = b_out @ w1g.T + b1                                # [1024]
    b2f = b2v + b_out @ wlin_m.T                            # [256]
    bias_h = np.concatenate([
        b_out.reshape(2, 128).T, b1f.reshape(8, 128).T, b2f.reshape(2, 128).T,
    ], axis=1).astype(np.float32)                           # [128, 12]
    shared = dict(wkv=wkv_h, wq=wq_h, wo=wo_h, w1t=w1t_h, w2t=w2t_h,
                  wlin=wlin_h, ident=ident_h, bias=bias_h)

    in_maps = []
    for b in range(B):
        xb = np.ascontiguousarray(x[b])
        in_maps.append(dict(x=xb.astype(NP_BF16), xt=np.ascontiguousarray(xb.T),
                            **shared))
    return in_maps


def kernel(**inputs):
    global LAST_RESULTS
    nc = _get_program()
    in_maps = _prep_maps(**inputs)
    res = run_bass_kernel_spmd(nc, in_maps, list(range(N_CORES)))
    LAST_RESULTS = res
    out = np.empty((B, N, C), np.float32)
    for b in range(B):
        yt = np.asarray(res.results[b]["yt"])    # [2, 128, N]
        out[b] = yt.reshape(C, N).T
    return out

